# revision 5
# baseline (speedup 1.0000x reference)
"""3-layer GCN (Kipf GraphConvolution) on 8 Trainium2 NeuronCores.

Math per layer: h = relu(adj @ (h @ W) + b); final out = relu(h3 + x).

Strategy (row-shard / 1D node partition):
  - adj is pre-transposed on the host; core c gets adjT[:, c*NS:(c+1)*NS]
    (i.e. the rows of adj it owns, laid out contraction-major) in bf16.
  - On-chip, activations are kept TRANSPOSED: hT [F, nodes] with features on
    partitions. The aggregation y = adj @ s is computed as
    yT = sT_stationary.T-contracted with adjT_moving:
        yT[f, i] = sum_j s[j, f] * adjT[j, i]
    so adj streams through the PE as the moving operand (N cycles per
    128x512 tile) and is never transposed on chip.
  - s tiles (normal layout [j, f]) are built from hT via a second small
    matmul: s[j, g] = sum_f hT[f, j] * W[f, g]  (lhsT = hT slice).
  - Layer boundaries need full-graph support: AllGather of the local
    hT [F, NS] (100 KB f32) across the 8 cores, twice (after layers 1, 2).
  - Layer 1's support s1 = x @ W1 is computed redundantly on every core from
    the full xT (no AllGather needed).

Per-core HBM traffic is dominated by streaming the 10000x1250 adjT shard
three times (bf16: 3 x 25 MB).
"""

import math
import os

import numpy as np

import concourse.bacc as bacc
import concourse.mybir as mybir
import concourse.tile as tile
from concourse.bass_utils import run_bass_kernel_spmd

# Problem geometry (hardcoded per the harness contract).
N = 10000
D_IN = 128
F1 = 20
F2 = 20
D_OUT = 128
NCORES = 8
NS = N // NCORES  # 1250 nodes per core
NT = math.ceil(N / 128)  # 79 contraction tiles, last has 16 rows
ICHUNK = 512

F32 = mybir.dt.float32
ADJ_DT = mybir.dt.bfloat16  # dtype adj (and s tiles) are streamed/matmul'd in

# Filled by kernel() so a harness/test can inspect HW timing.
LAST_RESULTS = None


def _chunks(total, step):
    out = []
    i = 0
    while i < total:
        out.append((i, min(step, total - i)))
        i += step
    return out


def build_program(n=N, ncores=NCORES, adj_dt=ADJ_DT):
    ns = n // ncores
    nt = math.ceil(n / 128)
    chunks = _chunks(ns, ICHUNK)
    relu = mybir.ActivationFunctionType.Relu

    nc = bacc.Bacc("TRN2", target_bir_lowering=False, debug=False)

    adjT = nc.dram_tensor("adjT", [n, ns], adj_dt, kind="ExternalInput")
    xT = nc.dram_tensor("xT", [D_IN, n], F32, kind="ExternalInput")
    xTs = nc.dram_tensor("xTs", [D_IN, ns], F32, kind="ExternalInput")
    W1 = nc.dram_tensor("W1", [D_IN, F1], F32, kind="ExternalInput")
    W2 = nc.dram_tensor("W2", [F1, F2], F32, kind="ExternalInput")
    W3 = nc.dram_tensor("W3", [F2, D_OUT], F32, kind="ExternalInput")
    b1 = nc.dram_tensor("b1", [F1, 1], F32, kind="ExternalInput")
    b2 = nc.dram_tensor("b2", [F2, 1], F32, kind="ExternalInput")
    b3 = nc.dram_tensor("b3", [D_OUT, 1], F32, kind="ExternalInput")
    outT = nc.dram_tensor("outT", [D_OUT, ns], F32, kind="ExternalOutput")

    with tile.TileContext(nc, num_cores=ncores) as tc:
        with (
            tc.tile_pool(name="const", bufs=1) as const,
            tc.tile_pool(name="s", bufs=1) as spool,
            tc.tile_pool(name="h", bufs=1) as hpool,
            tc.tile_pool(name="adj", bufs=6) as adjpool,
            tc.tile_pool(name="psy", bufs=2, space="PSUM") as psy_pool,
            tc.tile_pool(name="pss", bufs=2, space="PSUM") as pss_pool,
            tc.tile_pool(name="dram", bufs=1, space="DRAM") as dpool,
        ):
            w1_sb = const.tile([D_IN, F1], F32, tag="w1")
            w2_sb = const.tile([F1, F2], F32, tag="w2")
            w3_sb = const.tile([F2, D_OUT], F32, tag="w3")
            b1_sb = const.tile([F1, 1], F32, tag="b1")
            b2_sb = const.tile([F2, 1], F32, tag="b2")
            b3_sb = const.tile([D_OUT, 1], F32, tag="b3")
            xts_sb = const.tile([D_IN, ns], F32, tag="xts")
            for sb, dr in (
                (w1_sb, W1),
                (w2_sb, W2),
                (w3_sb, W3),
                (b1_sb, b1),
                (b2_sb, b2),
                (b3_sb, b3),
                (xts_sb, xTs),
            ):
                nc.sync.dma_start(out=sb[:, :], in_=dr[:, :])

            def build_support(src_sb, w_sb, fin, fout, s_sb):
                """s[j, g] = sum_f src_T[f, j] * W[f, g], tiled over j."""
                for jt in range(nt):
                    m = min(128, n - jt * 128)
                    ps = pss_pool.tile([128, max(F1, D_OUT)], F32, tag="pss")
                    nc.tensor.matmul(
                        ps[:m, :fout],
                        lhsT=src_sb[:fin, jt * 128 : jt * 128 + m],
                        rhs=w_sb[:fin, :fout],
                        start=True,
                        stop=True,
                    )
                    nc.vector.tensor_copy(
                        s_sb[:m, jt * fout : (jt + 1) * fout], ps[:m, :fout]
                    )

            def aggregate(s_sb, fout):
                """yT[f, i] += s_tile.T @ adjT_tile over all contraction tiles."""
                psy = [
                    psy_pool.tile(
                        [128, ICHUNK], F32, tag=f"psy{ic}", name=f"psy{ic}"
                    )
                    for ic in range(len(chunks))
                ]
                for kt in range(nt):
                    k = min(128, n - kt * 128)
                    at = adjpool.tile([128, ns], adj_dt, tag="adjstream")
                    nc.sync.dma_start(
                        out=at[:k, :], in_=adjT[kt * 128 : kt * 128 + k, :]
                    )
                    for ic, (i0, ilen) in enumerate(chunks):
                        nc.tensor.matmul(
                            psy[ic][:fout, :ilen],
                            lhsT=s_sb[:k, kt * fout : (kt + 1) * fout],
                            rhs=at[:k, i0 : i0 + ilen],
                            start=(kt == 0),
                            stop=(kt == nt - 1),
                        )
                return psy

            def relu_bias(psy, b_sb, fout, dst_sb):
                for ic, (i0, ilen) in enumerate(chunks):
                    nc.scalar.activation(
                        dst_sb[:fout, i0 : i0 + ilen],
                        psy[ic][:fout, :ilen],
                        relu,
                        bias=b_sb[:fout, :],
                    )

            def allgather_h(h_loc, f, layer):
                cc_in = dpool.tile([f, ns], F32, tag=f"ccin{layer}")
                cc_out = dpool.tile([ncores * f, ns], F32, tag=f"ccout{layer}")
                nc.sync.dma_start(out=cc_in[:, :], in_=h_loc[:, :])
                nc.gpsimd.collective_compute(
                    "AllGather",
                    mybir.AluOpType.bypass,
                    replica_groups=[list(range(ncores))],
                    ins=[cc_in.opt()],
                    outs=[cc_out.opt()],
                )
                h_full = hpool.tile([f, n], F32, tag=f"hfull{layer}")
                nc.sync.dma_start(
                    out=h_full[:, :].rearrange("p (r i) -> p r i", r=ncores),
                    in_=cc_out[:, :].rearrange("(r p) i -> p r i", p=f),
                )
                return h_full

            # ---- Layer 1: s1 = x @ W1 built redundantly from full xT ----
            xt_sb = const.tile([D_IN, n], F32, tag="xt")
            nc.sync.dma_start(out=xt_sb[:, :], in_=xT[:, :])
            s1_sb = spool.tile([128, nt * F1], adj_dt, tag="s1")
            build_support(xt_sb, w1_sb, D_IN, F1, s1_sb)
            psy1 = aggregate(s1_sb, F1)
            h1_loc = hpool.tile([F1, ns], F32, tag="hloc1")
            relu_bias(psy1, b1_sb, F1, h1_loc)
            h1_full = allgather_h(h1_loc, F1, 1)

            # ---- Layer 2 ----
            s2_sb = spool.tile([128, nt * F2], adj_dt, tag="s2")
            build_support(h1_full, w2_sb, F1, F2, s2_sb)
            psy2 = aggregate(s2_sb, F2)
            h2_loc = hpool.tile([F2, ns], F32, tag="hloc2")
            relu_bias(psy2, b2_sb, F2, h2_loc)
            h2_full = allgather_h(h2_loc, F2, 2)

            # ---- Layer 3 ----
            s3_sb = spool.tile([128, nt * D_OUT], adj_dt, tag="s3")
            build_support(h2_full, w3_sb, F2, D_OUT, s3_sb)
            psy3 = aggregate(s3_sb, D_OUT)
            h3_sb = hpool.tile([D_OUT, ns], F32, tag="h3")
            relu_bias(psy3, b3_sb, D_OUT, h3_sb)

            # ---- out = relu(h3 + x) ----
            o_sb = hpool.tile([D_OUT, ns], F32, tag="osum")
            nc.vector.tensor_add(o_sb[:, :], h3_sb[:, :], xts_sb[:, :])
            r_sb = hpool.tile([D_OUT, ns], F32, tag="orelu")
            nc.vector.tensor_relu(r_sb[:, :], o_sb[:, :])
            nc.sync.dma_start(out=outT[:, :], in_=r_sb[:, :])

    nc.compile()
    return nc


def _ensure_ntff_hook():
    """Register the axon NTFF profile hook if the image's antenv lacks it.

    Mirrors trn_agent_boot.trn_boot._ntff_profile_via_ctypes — drives NRT
    profiling through libaxon_pjrt.so's C ABI so run_bass_kernel_spmd can
    capture exec_time_ns under axon. Only used when tracing is requested.
    """
    import contextlib
    import ctypes
    import sys
    import types

    try:
        from antenv.axon_hooks import get_axon_ntff_profile_hook  # noqa: F401

        return
    except ImportError:
        pass

    so_path = "/opt/axon/libaxon_pjrt.so"
    lib = ctypes.CDLL(so_path)
    if not hasattr(lib, "axon_start_nrt_profile"):
        return
    lib.axon_start_nrt_profile.argtypes = [
        ctypes.POINTER(ctypes.c_int64),
        ctypes.c_size_t,
    ]
    lib.axon_start_nrt_profile.restype = ctypes.c_int64
    lib.axon_stop_nrt_profile.argtypes = [ctypes.c_char_p]
    lib.axon_stop_nrt_profile.restype = ctypes.c_int64

    @contextlib.contextmanager
    def _hook(output_dir, device_ids):
        import jax

        jax.devices()
        if device_ids:
            ids = (ctypes.c_int64 * len(device_ids))(*device_ids)
            rc = lib.axon_start_nrt_profile(ids, len(device_ids))
        else:
            rc = lib.axon_start_nrt_profile(None, 0)
        if rc != 0:
            raise RuntimeError(f"axon_start_nrt_profile rc={rc}")
        try:
            yield
        finally:
            n = lib.axon_stop_nrt_profile(str(output_dir).encode())
            print(f"ntff profile: {n} file(s) written to {output_dir}")

    mod = types.ModuleType("antenv.axon_hooks")
    _state = {"hook": _hook}
    mod.get_axon_ntff_profile_hook = lambda: _state["hook"]
    mod.set_axon_ntff_profile_hook = lambda h: _state.update(hook=h)
    sys.modules["antenv.axon_hooks"] = mod
    import antenv

    antenv.axon_hooks = mod


_PROGRAM = None


def _get_program():
    global _PROGRAM
    if _PROGRAM is None:
        _PROGRAM = build_program()
    return _PROGRAM


def kernel(**inputs):
    global LAST_RESULTS
    x = np.asarray(inputs["x"], dtype=np.float32)
    adj = np.asarray(inputs["adj"], dtype=np.float32)
    np_adj_dt = mybir.dt.np(ADJ_DT)

    adjT = np.ascontiguousarray(adj.T).astype(np_adj_dt)
    xT = np.ascontiguousarray(x.T)
    base = {
        "xT": xT,
        "W1": np.asarray(inputs["W1"], np.float32),
        "W2": np.asarray(inputs["W2"], np.float32),
        "W3": np.asarray(inputs["W3"], np.float32),
        "b1": np.asarray(inputs["b1"], np.float32).reshape(F1, 1),
        "b2": np.asarray(inputs["b2"], np.float32).reshape(F2, 1),
        "b3": np.asarray(inputs["b3"], np.float32).reshape(D_OUT, 1),
    }
    in_maps = []
    for c in range(NCORES):
        sl = slice(c * NS, (c + 1) * NS)
        in_maps.append(
            dict(
                base,
                adjT=np.ascontiguousarray(adjT[:, sl]),
                xTs=np.ascontiguousarray(xT[:, sl]),
            )
        )

    nc = _get_program()
    trace = bool(int(os.environ.get("GCN_TRACE", "0")))
    extra = {}
    if trace:
        _ensure_ntff_hook()
        if os.environ.get("GCN_TRACE_DIR"):
            os.makedirs(os.environ["GCN_TRACE_DIR"], exist_ok=True)
            extra["tmpdir"] = os.environ["GCN_TRACE_DIR"]
    LAST_RESULTS = run_bass_kernel_spmd(
        nc, in_maps, list(range(NCORES)), trace=trace, **extra
    )
    out = np.concatenate(
        [np.asarray(LAST_RESULTS.results[c]["outT"]).T for c in range(NCORES)],
        axis=0,
    )
    return np.ascontiguousarray(out.astype(np.float32))


# revision 14
# speedup vs baseline: 1.2749x; 1.2749x over previous
"""3-layer GCN (Kipf GraphConvolution) on 8 Trainium2 NeuronCores.

Math per layer: h = relu(adj @ (h @ W) + b); final out = relu(h3 + x).

Strategy (row-shard / 1D node partition):
  - adj is pre-transposed on the host; core c gets adjT[:, c*NS:(c+1)*NS]
    (i.e. the rows of adj it owns, laid out contraction-major) in bf16.
  - On-chip, activations are kept TRANSPOSED: hT [F, nodes] with features on
    partitions. The aggregation y = adj @ s is computed as
    yT = sT_stationary.T-contracted with adjT_moving:
        yT[f, i] = sum_j s[j, f] * adjT[j, i]
    so adj streams through the PE as the moving operand (N cycles per
    128x512 tile) and is never transposed on chip.
  - s tiles (normal layout [j, f]) are built from hT via a second small
    matmul: s[j, g] = sum_f hT[f, j] * W[f, g]  (lhsT = hT slice).
  - Layer boundaries need full-graph support: AllGather of the local
    hT [F, NS] (100 KB f32) across the 8 cores, twice (after layers 1, 2).
  - Layer 1's support s1 = x @ W1 is computed redundantly on every core from
    the full xT (no AllGather needed).

Per-core HBM traffic is dominated by streaming the 10000x1250 adjT shard
three times (bf16: 3 x 25 MB).
"""

import math
import os

import numpy as np

import concourse.bacc as bacc
import concourse.mybir as mybir
import concourse.tile as tile
from concourse.bass_utils import run_bass_kernel_spmd

# Problem geometry (hardcoded per the harness contract).
N = 10000
D_IN = 128
F1 = 20
F2 = 20
D_OUT = 128
NCORES = 8
NS = N // NCORES  # 1250 nodes per core
NT = math.ceil(N / 128)  # 79 contraction tiles, last has 16 rows
ICHUNK = 512

F32 = mybir.dt.float32
ADJ_DT = mybir.dt.bfloat16  # dtype adj (and s tiles) are streamed/matmul'd in

# Filled by kernel() so a harness/test can inspect HW timing.
LAST_RESULTS = None


def _chunks(total, step):
    out = []
    i = 0
    while i < total:
        out.append((i, min(step, total - i)))
        i += step
    return out


def build_program(n=N, ncores=NCORES, adj_dt=ADJ_DT):
    ns = n // ncores
    nt = math.ceil(n / 128)
    chunks = _chunks(ns, ICHUNK)
    relu = mybir.ActivationFunctionType.Relu
    # adjT k-tiles are streamed in slabs of SLAB tiles per DMA (~1.25 MB each)
    SLAB = 4
    n_full_slabs = (n // 128) // SLAB  # full 4x128-row slabs
    nt_tail_start = n_full_slabs * SLAB  # remaining k-tiles loaded singly

    nc = bacc.Bacc("TRN2", target_bir_lowering=False, debug=False)

    adjT = nc.dram_tensor("adjT", [n, ns], adj_dt, kind="ExternalInput")
    xT = nc.dram_tensor("xT", [D_IN, n], adj_dt, kind="ExternalInput")
    xTs = nc.dram_tensor("xTs", [D_IN, ns], F32, kind="ExternalInput")
    W1 = nc.dram_tensor("W1", [D_IN, F1], adj_dt, kind="ExternalInput")
    W2 = nc.dram_tensor("W2", [F1, F2], adj_dt, kind="ExternalInput")
    W3 = nc.dram_tensor("W3", [F2, D_OUT], adj_dt, kind="ExternalInput")
    b1 = nc.dram_tensor("b1", [F1, 1], F32, kind="ExternalInput")
    b2 = nc.dram_tensor("b2", [F2, 1], F32, kind="ExternalInput")
    b3 = nc.dram_tensor("b3", [D_OUT, 1], F32, kind="ExternalInput")
    outT = nc.dram_tensor("outT", [D_OUT, ns], F32, kind="ExternalOutput")

    with tile.TileContext(nc, num_cores=ncores) as tc:
        with (
            tc.tile_pool(name="const", bufs=1) as const,
            tc.tile_pool(name="s", bufs=1) as spool,
            tc.tile_pool(name="h", bufs=1) as hpool,
            tc.tile_pool(name="adj", bufs=3) as adjpool,
            tc.tile_pool(name="psy", bufs=2, space="PSUM") as psy_pool,
            tc.tile_pool(name="pss", bufs=2, space="PSUM") as pss_pool,
            tc.tile_pool(name="dram", bufs=1, space="DRAM") as dpool,
        ):
            w1_sb = const.tile([D_IN, F1], adj_dt, tag="w1")
            w2_sb = const.tile([F1, F2], adj_dt, tag="w2")
            w3_sb = const.tile([F2, D_OUT], adj_dt, tag="w3")
            b1_sb = const.tile([F1, 1], F32, tag="b1")
            b2_sb = const.tile([F2, 1], F32, tag="b2")
            b3_sb = const.tile([D_OUT, 1], F32, tag="b3")
            xts_sb = const.tile([D_IN, ns], F32, tag="xts")
            for sb, dr in (
                (w1_sb, W1),
                (w2_sb, W2),
                (w3_sb, W3),
                (b1_sb, b1),
                (b2_sb, b2),
                (b3_sb, b3),
                (xts_sb, xTs),
            ):
                nc.sync.dma_start(out=sb[:, :], in_=dr[:, :])

            def build_support(src_sb, w_sb, fin, fout, s_sb):
                """s[j, g] = sum_f src_T[f, j] * W[f, g], tiled over j."""
                for jt in range(nt):
                    m = min(128, n - jt * 128)
                    ps = pss_pool.tile([128, max(F1, D_OUT)], F32, tag="pss")
                    nc.tensor.matmul(
                        ps[:m, :fout],
                        lhsT=src_sb[:fin, jt * 128 : jt * 128 + m],
                        rhs=w_sb[:fin, :fout],
                        start=True,
                        stop=True,
                    )
                    nc.vector.tensor_copy(
                        s_sb[:m, jt * fout : (jt + 1) * fout], ps[:m, :fout]
                    )

            def aggregate(s_sb, fout):
                """yT[f, i] += s_tile.T @ adjT_tile over all contraction tiles.

                adjT is streamed in slabs of SLAB k-tiles per DMA (row-blocks
                [512, ns] loaded as [128, SLAB*ns] with the 128-row sub-blocks
                side by side in the free dim) so each DMA is ~1.25 MB.
                """

                def mm(kt, at_slice, k):
                    for ic, (i0, ilen) in enumerate(chunks):
                        nc.tensor.matmul(
                            psy[ic][:fout, :ilen],
                            lhsT=s_sb[:k, kt * fout : (kt + 1) * fout],
                            rhs=at_slice[:k, i0 : i0 + ilen],
                            start=(kt == 0),
                            stop=(kt == nt - 1),
                        )

                psy = [
                    psy_pool.tile(
                        [128, ICHUNK], F32, tag=f"psy{ic}", name=f"psy{ic}"
                    )
                    for ic in range(len(chunks))
                ]
                for sl in range(n_full_slabs):
                    at = adjpool.tile([128, SLAB * ns], adj_dt, tag="adjstream")
                    r0 = sl * SLAB * 128
                    nc.sync.dma_start(
                        out=at[:, :].rearrange("p (a i) -> p a i", a=SLAB),
                        in_=adjT[r0 : r0 + SLAB * 128, :].rearrange(
                            "(a p) i -> p a i", p=128
                        ),
                    )
                    for a in range(SLAB):
                        mm(sl * SLAB + a, at[:, a * ns : (a + 1) * ns], 128)
                for kt in range(nt_tail_start, nt):
                    k = min(128, n - kt * 128)
                    at = adjpool.tile(
                        [128, SLAB * ns], adj_dt, tag="adjstream", name="at_tail"
                    )
                    nc.sync.dma_start(
                        out=at[:k, :ns], in_=adjT[kt * 128 : kt * 128 + k, :]
                    )
                    mm(kt, at[:, :ns], k)
                return psy

            def relu_bias(psy, b_sb, fout, dst_sb):
                for ic, (i0, ilen) in enumerate(chunks):
                    nc.scalar.activation(
                        dst_sb[:fout, i0 : i0 + ilen],
                        psy[ic][:fout, :ilen],
                        relu,
                        bias=b_sb[:fout, :],
                    )

            def allgather_h(h_loc, f, layer):
                cc_in = dpool.tile([f, ns], adj_dt, tag=f"ccin{layer}")
                cc_out = dpool.tile([ncores * f, ns], adj_dt, tag=f"ccout{layer}")
                nc.sync.dma_start(out=cc_in[:, :], in_=h_loc[:, :])
                nc.gpsimd.collective_compute(
                    "AllGather",
                    mybir.AluOpType.bypass,
                    replica_groups=[list(range(ncores))],
                    ins=[cc_in.opt()],
                    outs=[cc_out.opt()],
                )
                h_full = hpool.tile([f, n], adj_dt, tag=f"hfull{layer}")
                nc.sync.dma_start(
                    out=h_full[:, :].rearrange("p (r i) -> p r i", r=ncores),
                    in_=cc_out[:, :].rearrange("(r p) i -> p r i", p=f),
                )
                return h_full

            # ---- Layer 1: s1 = x @ W1 built redundantly from full xT ----
            xt_sb = const.tile([D_IN, n], adj_dt, tag="xt")
            nc.sync.dma_start(out=xt_sb[:, :], in_=xT[:, :])
            s1_sb = spool.tile([128, nt * F1], adj_dt, tag="s1")
            build_support(xt_sb, w1_sb, D_IN, F1, s1_sb)
            psy1 = aggregate(s1_sb, F1)
            h1_loc = hpool.tile([F1, ns], adj_dt, tag="hloc1")
            relu_bias(psy1, b1_sb, F1, h1_loc)
            h1_full = allgather_h(h1_loc, F1, 1)

            # ---- Layer 2 ----
            s2_sb = spool.tile([128, nt * F2], adj_dt, tag="s2")
            build_support(h1_full, w2_sb, F1, F2, s2_sb)
            psy2 = aggregate(s2_sb, F2)
            h2_loc = hpool.tile([F2, ns], adj_dt, tag="hloc2")
            relu_bias(psy2, b2_sb, F2, h2_loc)
            h2_full = allgather_h(h2_loc, F2, 2)

            # ---- Layer 3 ----
            s3_sb = spool.tile([128, nt * D_OUT], adj_dt, tag="s3")
            build_support(h2_full, w3_sb, F2, D_OUT, s3_sb)
            psy3 = aggregate(s3_sb, D_OUT)
            h3_sb = hpool.tile([D_OUT, ns], F32, tag="h3")
            relu_bias(psy3, b3_sb, D_OUT, h3_sb)

            # ---- out = relu(h3 + x) ----
            o_sb = hpool.tile([D_OUT, ns], F32, tag="osum")
            nc.vector.tensor_add(o_sb[:, :], h3_sb[:, :], xts_sb[:, :])
            r_sb = hpool.tile([D_OUT, ns], F32, tag="orelu")
            nc.vector.tensor_relu(r_sb[:, :], o_sb[:, :])
            nc.sync.dma_start(out=outT[:, :], in_=r_sb[:, :])

    nc.compile()
    return nc


def _ensure_ntff_hook():
    """Register the axon NTFF profile hook if the image's antenv lacks it.

    Mirrors trn_agent_boot.trn_boot._ntff_profile_via_ctypes — drives NRT
    profiling through libaxon_pjrt.so's C ABI so run_bass_kernel_spmd can
    capture exec_time_ns under axon. Only used when tracing is requested.
    """
    import contextlib
    import ctypes
    import sys
    import types

    try:
        from antenv.axon_hooks import get_axon_ntff_profile_hook  # noqa: F401

        return
    except ImportError:
        pass

    so_path = "/opt/axon/libaxon_pjrt.so"
    lib = ctypes.CDLL(so_path)
    if not hasattr(lib, "axon_start_nrt_profile"):
        return
    lib.axon_start_nrt_profile.argtypes = [
        ctypes.POINTER(ctypes.c_int64),
        ctypes.c_size_t,
    ]
    lib.axon_start_nrt_profile.restype = ctypes.c_int64
    lib.axon_stop_nrt_profile.argtypes = [ctypes.c_char_p]
    lib.axon_stop_nrt_profile.restype = ctypes.c_int64

    @contextlib.contextmanager
    def _hook(output_dir, device_ids):
        import jax

        jax.devices()
        if device_ids:
            ids = (ctypes.c_int64 * len(device_ids))(*device_ids)
            rc = lib.axon_start_nrt_profile(ids, len(device_ids))
        else:
            rc = lib.axon_start_nrt_profile(None, 0)
        if rc != 0:
            raise RuntimeError(f"axon_start_nrt_profile rc={rc}")
        try:
            yield
        finally:
            n = lib.axon_stop_nrt_profile(str(output_dir).encode())
            print(f"ntff profile: {n} file(s) written to {output_dir}")

    mod = types.ModuleType("antenv.axon_hooks")
    _state = {"hook": _hook}
    mod.get_axon_ntff_profile_hook = lambda: _state["hook"]
    mod.set_axon_ntff_profile_hook = lambda h: _state.update(hook=h)
    sys.modules["antenv.axon_hooks"] = mod
    import antenv

    antenv.axon_hooks = mod


_PROGRAM = None


def _get_program():
    global _PROGRAM
    if _PROGRAM is None:
        _PROGRAM = build_program()
    return _PROGRAM


def kernel(**inputs):
    global LAST_RESULTS
    x = np.asarray(inputs["x"], dtype=np.float32)
    adj = np.asarray(inputs["adj"], dtype=np.float32)
    np_adj_dt = mybir.dt.np(ADJ_DT)

    adjT = np.ascontiguousarray(adj.T).astype(np_adj_dt)
    xT = np.ascontiguousarray(x.T)
    base = {
        "xT": xT.astype(np_adj_dt),
        "W1": np.asarray(inputs["W1"], np.float32).astype(np_adj_dt),
        "W2": np.asarray(inputs["W2"], np.float32).astype(np_adj_dt),
        "W3": np.asarray(inputs["W3"], np.float32).astype(np_adj_dt),
        "b1": np.asarray(inputs["b1"], np.float32).reshape(F1, 1),
        "b2": np.asarray(inputs["b2"], np.float32).reshape(F2, 1),
        "b3": np.asarray(inputs["b3"], np.float32).reshape(D_OUT, 1),
    }
    in_maps = []
    for c in range(NCORES):
        sl = slice(c * NS, (c + 1) * NS)
        in_maps.append(
            dict(
                base,
                adjT=np.ascontiguousarray(adjT[:, sl]),
                xTs=np.ascontiguousarray(xT[:, sl]),
            )
        )

    nc = _get_program()
    trace = bool(int(os.environ.get("GCN_TRACE", "0")))
    extra = {}
    if trace:
        _ensure_ntff_hook()
        if os.environ.get("GCN_TRACE_DIR"):
            os.makedirs(os.environ["GCN_TRACE_DIR"], exist_ok=True)
            extra["tmpdir"] = os.environ["GCN_TRACE_DIR"]
    LAST_RESULTS = run_bass_kernel_spmd(
        nc, in_maps, list(range(NCORES)), trace=trace, **extra
    )
    out = np.concatenate(
        [np.asarray(LAST_RESULTS.results[c]["outT"]).T for c in range(NCORES)],
        axis=0,
    )
    return np.ascontiguousarray(out.astype(np.float32))


# revision 17
# speedup vs baseline: 1.3675x; 1.0726x over previous
"""3-layer GCN (Kipf GraphConvolution) on 8 Trainium2 NeuronCores.

Math per layer: h = relu(adj @ (h @ W) + b); final out = relu(h3 + x).

Strategy (row-shard / 1D node partition):
  - adj is pre-transposed on the host; core c gets adjT[:, c*NS:(c+1)*NS]
    (i.e. the rows of adj it owns, laid out contraction-major) in bf16.
  - On-chip, activations are kept TRANSPOSED: hT [F, nodes] with features on
    partitions. The aggregation y = adj @ s is computed as
    yT = sT_stationary.T-contracted with adjT_moving:
        yT[f, i] = sum_j s[j, f] * adjT[j, i]
    so adj streams through the PE as the moving operand (N cycles per
    128x512 tile) and is never transposed on chip.
  - s tiles (normal layout [j, f]) are built from hT via a second small
    matmul: s[j, g] = sum_f hT[f, j] * W[f, g]  (lhsT = hT slice).
  - Layer boundaries need full-graph support: AllGather of the local
    hT [F, NS] (100 KB f32) across the 8 cores, twice (after layers 1, 2).
  - Layer 1's support s1 = x @ W1 is computed redundantly on every core from
    the full xT (no AllGather needed).

Per-core HBM traffic is dominated by streaming the 10000x1250 adjT shard
three times (bf16: 3 x 25 MB).
"""

import math
import os

import numpy as np

import concourse.bacc as bacc
import concourse.mybir as mybir
import concourse.tile as tile
from concourse.bass_utils import run_bass_kernel_spmd

# Problem geometry (hardcoded per the harness contract).
N = 10000
D_IN = 128
F1 = 20
F2 = 20
D_OUT = 128
NCORES = 8
NS = N // NCORES  # 1250 nodes per core
NT = math.ceil(N / 128)  # 79 contraction tiles, last has 16 rows
ICHUNK = 512

F32 = mybir.dt.float32
ADJ_DT = mybir.dt.bfloat16  # dtype adj (and s tiles) are streamed/matmul'd in

# Filled by kernel() so a harness/test can inspect HW timing.
LAST_RESULTS = None


def _chunks(total, step):
    out = []
    i = 0
    while i < total:
        out.append((i, min(step, total - i)))
        i += step
    return out


def build_program(n=N, ncores=NCORES, adj_dt=ADJ_DT):
    ns = n // ncores
    nt = math.ceil(n / 128)
    chunks = _chunks(ns, ICHUNK)
    relu = mybir.ActivationFunctionType.Relu
    # adjT k-tiles are streamed in slabs of SLAB tiles per DMA (~1.25 MB each)
    SLAB = 4
    n_full_slabs = (n // 128) // SLAB  # full 4x128-row slabs
    nt_tail_start = n_full_slabs * SLAB  # remaining k-tiles loaded singly

    nc = bacc.Bacc("TRN2", target_bir_lowering=False, debug=False)

    adjT = nc.dram_tensor("adjT", [n, ns], adj_dt, kind="ExternalInput")
    xT = nc.dram_tensor("xT", [D_IN, n], adj_dt, kind="ExternalInput")
    xTs = nc.dram_tensor("xTs", [D_IN, ns], F32, kind="ExternalInput")
    W1 = nc.dram_tensor("W1", [D_IN, F1], adj_dt, kind="ExternalInput")
    W2 = nc.dram_tensor("W2", [F1, F2], adj_dt, kind="ExternalInput")
    W3 = nc.dram_tensor("W3", [F2, D_OUT], adj_dt, kind="ExternalInput")
    b1 = nc.dram_tensor("b1", [F1, 1], F32, kind="ExternalInput")
    b2 = nc.dram_tensor("b2", [F2, 1], F32, kind="ExternalInput")
    b3 = nc.dram_tensor("b3", [D_OUT, 1], F32, kind="ExternalInput")
    outT = nc.dram_tensor("outT", [D_OUT, ns], F32, kind="ExternalOutput")

    with tile.TileContext(nc, num_cores=ncores) as tc:
        with (
            tc.tile_pool(name="const", bufs=1) as const,
            tc.tile_pool(name="s", bufs=1) as spool,
            tc.tile_pool(name="h", bufs=1) as hpool,
            tc.tile_pool(name="adj", bufs=5) as adjpool,
            tc.tile_pool(name="psy", bufs=2, space="PSUM") as psy_pool,
            tc.tile_pool(name="pss", bufs=2, space="PSUM") as pss_pool,
            tc.tile_pool(name="dram", bufs=1, space="DRAM") as dpool,
        ):
            w1_sb = const.tile([D_IN, F1], adj_dt, tag="w1")
            w2_sb = const.tile([F1, F2], adj_dt, tag="w2")
            w3_sb = const.tile([F2, D_OUT], adj_dt, tag="w3")
            b1_sb = const.tile([F1, 1], F32, tag="b1")
            b2_sb = const.tile([F2, 1], F32, tag="b2")
            b3_sb = const.tile([D_OUT, 1], F32, tag="b3")
            xts_sb = const.tile([D_IN, ns], F32, tag="xts")
            for sb, dr in (
                (w1_sb, W1),
                (w2_sb, W2),
                (w3_sb, W3),
                (b1_sb, b1),
                (b2_sb, b2),
                (b3_sb, b3),
                (xts_sb, xTs),
            ):
                nc.sync.dma_start(out=sb[:, :], in_=dr[:, :])

            def build_support(src_sb, w_sb, fin, fout, lname):
                """s[j, g] = sum_f src_T[f, j] * W[f, g], one tile per j-tile
                so the aggregation can consume tiles as they are built."""
                tiles = []
                for jt in range(nt):
                    m = min(128, n - jt * 128)
                    ps = pss_pool.tile([128, max(F1, D_OUT)], F32, tag="pss")
                    nc.tensor.matmul(
                        ps[:m, :fout],
                        lhsT=src_sb[:fin, jt * 128 : jt * 128 + m],
                        rhs=w_sb[:fin, :fout],
                        start=True,
                        stop=True,
                    )
                    st = spool.tile(
                        [128, fout], adj_dt, tag=f"{lname}_{jt}", name=f"{lname}_{jt}"
                    )
                    nc.vector.tensor_copy(st[:m, :], ps[:m, :fout])
                    tiles.append(st)
                return tiles

            def aggregate(s_tiles, fout):
                """yT[f, i] += s_tile.T @ adjT_tile over all contraction tiles.

                adjT is streamed in slabs of SLAB k-tiles per DMA (row-blocks
                [512, ns] loaded as [128, SLAB*ns] with the 128-row sub-blocks
                side by side in the free dim) so each DMA is ~1.25 MB.
                """

                def mm(kt, at_slice, k):
                    for ic, (i0, ilen) in enumerate(chunks):
                        nc.tensor.matmul(
                            psy[ic][:fout, :ilen],
                            lhsT=s_tiles[kt][:k, :fout],
                            rhs=at_slice[:k, i0 : i0 + ilen],
                            start=(kt == 0),
                            stop=(kt == nt - 1),
                        )

                psy = [
                    psy_pool.tile(
                        [128, ICHUNK], F32, tag=f"psy{ic}", name=f"psy{ic}"
                    )
                    for ic in range(len(chunks))
                ]
                for sl in range(n_full_slabs):
                    at = adjpool.tile([128, SLAB * ns], adj_dt, tag="adjstream")
                    r0 = sl * SLAB * 128
                    nc.sync.dma_start(
                        out=at[:, :].rearrange("p (a i) -> p a i", a=SLAB),
                        in_=adjT[r0 : r0 + SLAB * 128, :].rearrange(
                            "(a p) i -> p a i", p=128
                        ),
                    )
                    for a in range(SLAB):
                        mm(sl * SLAB + a, at[:, a * ns : (a + 1) * ns], 128)
                for kt in range(nt_tail_start, nt):
                    k = min(128, n - kt * 128)
                    at = adjpool.tile(
                        [128, SLAB * ns], adj_dt, tag="adjstream", name="at_tail"
                    )
                    nc.sync.dma_start(
                        out=at[:k, :ns], in_=adjT[kt * 128 : kt * 128 + k, :]
                    )
                    mm(kt, at[:, :ns], k)
                return psy

            def relu_bias(psy, b_sb, fout, dst_sb):
                for ic, (i0, ilen) in enumerate(chunks):
                    nc.scalar.activation(
                        dst_sb[:fout, i0 : i0 + ilen],
                        psy[ic][:fout, :ilen],
                        relu,
                        bias=b_sb[:fout, :],
                    )

            def allgather_h(h_loc, f, layer):
                cc_in = dpool.tile([f, ns], adj_dt, tag=f"ccin{layer}")
                cc_out = dpool.tile([ncores * f, ns], adj_dt, tag=f"ccout{layer}")
                nc.gpsimd.dma_start(out=cc_in[:, :], in_=h_loc[:, :])
                nc.gpsimd.collective_compute(
                    "AllGather",
                    mybir.AluOpType.bypass,
                    replica_groups=[list(range(ncores))],
                    ins=[cc_in.opt()],
                    outs=[cc_out.opt()],
                )
                h_full = hpool.tile([f, n], adj_dt, tag=f"hfull{layer}")
                nc.gpsimd.dma_start(
                    out=h_full[:, :].rearrange("p (r i) -> p r i", r=ncores),
                    in_=cc_out[:, :].rearrange("(r p) i -> p r i", p=f),
                )
                return h_full

            # ---- Layer 1: s1 = x @ W1 built redundantly from full xT ----
            xt_sb = const.tile([D_IN, n], adj_dt, tag="xt")
            nc.sync.dma_start(out=xt_sb[:, :], in_=xT[:, :])
            s1_tiles = build_support(xt_sb, w1_sb, D_IN, F1, "s1")
            psy1 = aggregate(s1_tiles, F1)
            h1_loc = hpool.tile([F1, ns], adj_dt, tag="hloc1")
            relu_bias(psy1, b1_sb, F1, h1_loc)
            h1_full = allgather_h(h1_loc, F1, 1)

            # ---- Layer 2 ----
            s2_tiles = build_support(h1_full, w2_sb, F1, F2, "s2")
            psy2 = aggregate(s2_tiles, F2)
            h2_loc = hpool.tile([F2, ns], adj_dt, tag="hloc2")
            relu_bias(psy2, b2_sb, F2, h2_loc)
            h2_full = allgather_h(h2_loc, F2, 2)

            # ---- Layer 3 ----
            s3_tiles = build_support(h2_full, w3_sb, F2, D_OUT, "s3")
            psy3 = aggregate(s3_tiles, D_OUT)
            h3_sb = hpool.tile([D_OUT, ns], F32, tag="h3")
            relu_bias(psy3, b3_sb, D_OUT, h3_sb)

            # ---- out = relu(h3 + x) ----
            o_sb = hpool.tile([D_OUT, ns], F32, tag="osum")
            nc.vector.tensor_add(o_sb[:, :], h3_sb[:, :], xts_sb[:, :])
            r_sb = hpool.tile([D_OUT, ns], F32, tag="orelu")
            nc.vector.tensor_relu(r_sb[:, :], o_sb[:, :])
            nc.sync.dma_start(out=outT[:, :], in_=r_sb[:, :])

    nc.compile()
    return nc


def _ensure_ntff_hook():
    """Register the axon NTFF profile hook if the image's antenv lacks it.

    Mirrors trn_agent_boot.trn_boot._ntff_profile_via_ctypes — drives NRT
    profiling through libaxon_pjrt.so's C ABI so run_bass_kernel_spmd can
    capture exec_time_ns under axon. Only used when tracing is requested.
    """
    import contextlib
    import ctypes
    import sys
    import types

    try:
        from antenv.axon_hooks import get_axon_ntff_profile_hook  # noqa: F401

        return
    except ImportError:
        pass

    so_path = "/opt/axon/libaxon_pjrt.so"
    lib = ctypes.CDLL(so_path)
    if not hasattr(lib, "axon_start_nrt_profile"):
        return
    lib.axon_start_nrt_profile.argtypes = [
        ctypes.POINTER(ctypes.c_int64),
        ctypes.c_size_t,
    ]
    lib.axon_start_nrt_profile.restype = ctypes.c_int64
    lib.axon_stop_nrt_profile.argtypes = [ctypes.c_char_p]
    lib.axon_stop_nrt_profile.restype = ctypes.c_int64

    @contextlib.contextmanager
    def _hook(output_dir, device_ids):
        import jax

        jax.devices()
        if device_ids:
            ids = (ctypes.c_int64 * len(device_ids))(*device_ids)
            rc = lib.axon_start_nrt_profile(ids, len(device_ids))
        else:
            rc = lib.axon_start_nrt_profile(None, 0)
        if rc != 0:
            raise RuntimeError(f"axon_start_nrt_profile rc={rc}")
        try:
            yield
        finally:
            n = lib.axon_stop_nrt_profile(str(output_dir).encode())
            print(f"ntff profile: {n} file(s) written to {output_dir}")

    mod = types.ModuleType("antenv.axon_hooks")
    _state = {"hook": _hook}
    mod.get_axon_ntff_profile_hook = lambda: _state["hook"]
    mod.set_axon_ntff_profile_hook = lambda h: _state.update(hook=h)
    sys.modules["antenv.axon_hooks"] = mod
    import antenv

    antenv.axon_hooks = mod


_PROGRAM = None


def _get_program():
    global _PROGRAM
    if _PROGRAM is None:
        _PROGRAM = build_program()
    return _PROGRAM


def kernel(**inputs):
    global LAST_RESULTS
    x = np.asarray(inputs["x"], dtype=np.float32)
    adj = np.asarray(inputs["adj"], dtype=np.float32)
    np_adj_dt = mybir.dt.np(ADJ_DT)

    adjT = np.ascontiguousarray(adj.T).astype(np_adj_dt)
    xT = np.ascontiguousarray(x.T)
    base = {
        "xT": xT.astype(np_adj_dt),
        "W1": np.asarray(inputs["W1"], np.float32).astype(np_adj_dt),
        "W2": np.asarray(inputs["W2"], np.float32).astype(np_adj_dt),
        "W3": np.asarray(inputs["W3"], np.float32).astype(np_adj_dt),
        "b1": np.asarray(inputs["b1"], np.float32).reshape(F1, 1),
        "b2": np.asarray(inputs["b2"], np.float32).reshape(F2, 1),
        "b3": np.asarray(inputs["b3"], np.float32).reshape(D_OUT, 1),
    }
    in_maps = []
    for c in range(NCORES):
        sl = slice(c * NS, (c + 1) * NS)
        in_maps.append(
            dict(
                base,
                adjT=np.ascontiguousarray(adjT[:, sl]),
                xTs=np.ascontiguousarray(xT[:, sl]),
            )
        )

    nc = _get_program()
    trace = bool(int(os.environ.get("GCN_TRACE", "0")))
    extra = {}
    if trace:
        _ensure_ntff_hook()
        if os.environ.get("GCN_TRACE_DIR"):
            os.makedirs(os.environ["GCN_TRACE_DIR"], exist_ok=True)
            extra["tmpdir"] = os.environ["GCN_TRACE_DIR"]
    LAST_RESULTS = run_bass_kernel_spmd(
        nc, in_maps, list(range(NCORES)), trace=trace, **extra
    )
    out = np.concatenate(
        [np.asarray(LAST_RESULTS.results[c]["outT"]).T for c in range(NCORES)],
        axis=0,
    )
    return np.ascontiguousarray(out.astype(np.float32))


# revision 18
# speedup vs baseline: 1.4144x; 1.0343x over previous
"""3-layer GCN (Kipf GraphConvolution) on 8 Trainium2 NeuronCores.

Math per layer: h = relu(adj @ (h @ W) + b); final out = relu(h3 + x).

Strategy (row-shard / 1D node partition):
  - adj is pre-transposed on the host; core c gets adjT[:, c*NS:(c+1)*NS]
    (i.e. the rows of adj it owns, laid out contraction-major) in bf16.
  - On-chip, activations are kept TRANSPOSED: hT [F, nodes] with features on
    partitions. The aggregation y = adj @ s is computed as
    yT = sT_stationary.T-contracted with adjT_moving:
        yT[f, i] = sum_j s[j, f] * adjT[j, i]
    so adj streams through the PE as the moving operand (N cycles per
    128x512 tile) and is never transposed on chip.
  - s tiles (normal layout [j, f]) are built from hT via a second small
    matmul: s[j, g] = sum_f hT[f, j] * W[f, g]  (lhsT = hT slice).
  - Layer boundaries need full-graph support: AllGather of the local
    hT [F, NS] (100 KB f32) across the 8 cores, twice (after layers 1, 2).
  - Layer 1's support s1 = x @ W1 is computed redundantly on every core from
    the full xT (no AllGather needed).

Per-core HBM traffic is dominated by streaming the 10000x1250 adjT shard
three times (bf16: 3 x 25 MB).
"""

import math
import os

import numpy as np

import concourse.bacc as bacc
import concourse.mybir as mybir
import concourse.tile as tile
from concourse.bass_utils import run_bass_kernel_spmd

# Problem geometry (hardcoded per the harness contract).
N = 10000
D_IN = 128
F1 = 20
F2 = 20
D_OUT = 128
NCORES = 8
NS = N // NCORES  # 1250 nodes per core
NT = math.ceil(N / 128)  # 79 contraction tiles, last has 16 rows
ICHUNK = 512

F32 = mybir.dt.float32
ADJ_DT = mybir.dt.bfloat16  # dtype adj (and s tiles) are streamed/matmul'd in

# Filled by kernel() so a harness/test can inspect HW timing.
LAST_RESULTS = None


def _chunks(total, step):
    out = []
    i = 0
    while i < total:
        out.append((i, min(step, total - i)))
        i += step
    return out


def build_program(n=N, ncores=NCORES, adj_dt=ADJ_DT):
    ns = n // ncores
    nt = math.ceil(n / 128)
    chunks = _chunks(ns, ICHUNK)
    relu = mybir.ActivationFunctionType.Relu
    # adjT k-tiles are streamed in slabs of SLAB tiles per DMA (~640 KB each)
    SLAB = 2
    n_full_slabs = (n // 128) // SLAB  # full 4x128-row slabs
    nt_tail_start = n_full_slabs * SLAB  # remaining k-tiles loaded singly

    nc = bacc.Bacc("TRN2", target_bir_lowering=False, debug=False)

    adjT = nc.dram_tensor("adjT", [n, ns], adj_dt, kind="ExternalInput")
    xT = nc.dram_tensor("xT", [D_IN, n], adj_dt, kind="ExternalInput")
    xTs = nc.dram_tensor("xTs", [D_IN, ns], F32, kind="ExternalInput")
    W1 = nc.dram_tensor("W1", [D_IN, F1], adj_dt, kind="ExternalInput")
    W2 = nc.dram_tensor("W2", [F1, F2], adj_dt, kind="ExternalInput")
    W3 = nc.dram_tensor("W3", [F2, D_OUT], adj_dt, kind="ExternalInput")
    b1 = nc.dram_tensor("b1", [F1, 1], F32, kind="ExternalInput")
    b2 = nc.dram_tensor("b2", [F2, 1], F32, kind="ExternalInput")
    b3 = nc.dram_tensor("b3", [D_OUT, 1], F32, kind="ExternalInput")
    outT = nc.dram_tensor("outT", [D_OUT, ns], F32, kind="ExternalOutput")

    with tile.TileContext(nc, num_cores=ncores) as tc:
        with (
            tc.tile_pool(name="const", bufs=1) as const,
            tc.tile_pool(name="s", bufs=1) as spool,
            tc.tile_pool(name="h", bufs=1) as hpool,
            tc.tile_pool(name="adj", bufs=12) as adjpool,
            tc.tile_pool(name="psy", bufs=2, space="PSUM") as psy_pool,
            tc.tile_pool(name="pss", bufs=2, space="PSUM") as pss_pool,
            tc.tile_pool(name="dram", bufs=1, space="DRAM") as dpool,
        ):
            w1_sb = const.tile([D_IN, F1], adj_dt, tag="w1")
            w2_sb = const.tile([F1, F2], adj_dt, tag="w2")
            w3_sb = const.tile([F2, D_OUT], adj_dt, tag="w3")
            b1_sb = const.tile([F1, 1], F32, tag="b1")
            b2_sb = const.tile([F2, 1], F32, tag="b2")
            b3_sb = const.tile([D_OUT, 1], F32, tag="b3")
            xts_sb = const.tile([D_IN, ns], F32, tag="xts")
            for sb, dr in (
                (w1_sb, W1),
                (w2_sb, W2),
                (w3_sb, W3),
                (b1_sb, b1),
                (b2_sb, b2),
                (b3_sb, b3),
                (xts_sb, xTs),
            ):
                nc.gpsimd.dma_start(out=sb[:, :], in_=dr[:, :])

            def build_support(src_sb, w_sb, fin, fout, lname):
                """s[j, g] = sum_f src_T[f, j] * W[f, g], one tile per j-tile
                so the aggregation can consume tiles as they are built."""
                tiles = []
                for jt in range(nt):
                    m = min(128, n - jt * 128)
                    ps = pss_pool.tile([128, max(F1, D_OUT)], F32, tag="pss")
                    nc.tensor.matmul(
                        ps[:m, :fout],
                        lhsT=src_sb[:fin, jt * 128 : jt * 128 + m],
                        rhs=w_sb[:fin, :fout],
                        start=True,
                        stop=True,
                    )
                    st = spool.tile(
                        [128, fout], adj_dt, tag=f"{lname}_{jt}", name=f"{lname}_{jt}"
                    )
                    nc.vector.tensor_copy(st[:m, :], ps[:m, :fout])
                    tiles.append(st)
                return tiles

            def aggregate(s_tiles, fout):
                """yT[f, i] += s_tile.T @ adjT_tile over all contraction tiles.

                adjT is streamed in slabs of SLAB k-tiles per DMA (row-blocks
                [512, ns] loaded as [128, SLAB*ns] with the 128-row sub-blocks
                side by side in the free dim) so each DMA is ~1.25 MB.
                """

                def mm(kt, at_slice, k):
                    for ic, (i0, ilen) in enumerate(chunks):
                        nc.tensor.matmul(
                            psy[ic][:fout, :ilen],
                            lhsT=s_tiles[kt][:k, :fout],
                            rhs=at_slice[:k, i0 : i0 + ilen],
                            start=(kt == 0),
                            stop=(kt == nt - 1),
                        )

                psy = [
                    psy_pool.tile(
                        [128, ICHUNK], F32, tag=f"psy{ic}", name=f"psy{ic}"
                    )
                    for ic in range(len(chunks))
                ]
                for sl in range(n_full_slabs):
                    at = adjpool.tile([128, SLAB * ns], adj_dt, tag="adjstream")
                    r0 = sl * SLAB * 128
                    nc.sync.dma_start(
                        out=at[:, :].rearrange("p (a i) -> p a i", a=SLAB),
                        in_=adjT[r0 : r0 + SLAB * 128, :].rearrange(
                            "(a p) i -> p a i", p=128
                        ),
                    )
                    for a in range(SLAB):
                        mm(sl * SLAB + a, at[:, a * ns : (a + 1) * ns], 128)
                for kt in range(nt_tail_start, nt):
                    k = min(128, n - kt * 128)
                    at = adjpool.tile(
                        [128, SLAB * ns], adj_dt, tag="adjstream", name="at_tail"
                    )
                    nc.sync.dma_start(
                        out=at[:k, :ns], in_=adjT[kt * 128 : kt * 128 + k, :]
                    )
                    mm(kt, at[:, :ns], k)
                return psy

            def relu_bias(psy, b_sb, fout, dst_sb):
                for ic, (i0, ilen) in enumerate(chunks):
                    nc.scalar.activation(
                        dst_sb[:fout, i0 : i0 + ilen],
                        psy[ic][:fout, :ilen],
                        relu,
                        bias=b_sb[:fout, :],
                    )

            def allgather_h(h_loc, f, layer):
                cc_in = dpool.tile([f, ns], adj_dt, tag=f"ccin{layer}")
                cc_out = dpool.tile([ncores * f, ns], adj_dt, tag=f"ccout{layer}")
                nc.gpsimd.dma_start(out=cc_in[:, :], in_=h_loc[:, :])
                nc.gpsimd.collective_compute(
                    "AllGather",
                    mybir.AluOpType.bypass,
                    replica_groups=[list(range(ncores))],
                    ins=[cc_in.opt()],
                    outs=[cc_out.opt()],
                )
                h_full = hpool.tile([f, n], adj_dt, tag=f"hfull{layer}")
                nc.gpsimd.dma_start(
                    out=h_full[:, :].rearrange("p (r i) -> p r i", r=ncores),
                    in_=cc_out[:, :].rearrange("(r p) i -> p r i", p=f),
                )
                return h_full

            # ---- Layer 1: s1 = x @ W1 built redundantly from full xT ----
            xt_sb = const.tile([D_IN, n], adj_dt, tag="xt")
            nc.gpsimd.dma_start(out=xt_sb[:, :], in_=xT[:, :])
            s1_tiles = build_support(xt_sb, w1_sb, D_IN, F1, "s1")
            psy1 = aggregate(s1_tiles, F1)
            h1_loc = hpool.tile([F1, ns], adj_dt, tag="hloc1")
            relu_bias(psy1, b1_sb, F1, h1_loc)
            h1_full = allgather_h(h1_loc, F1, 1)

            # ---- Layer 2 ----
            s2_tiles = build_support(h1_full, w2_sb, F1, F2, "s2")
            psy2 = aggregate(s2_tiles, F2)
            h2_loc = hpool.tile([F2, ns], adj_dt, tag="hloc2")
            relu_bias(psy2, b2_sb, F2, h2_loc)
            h2_full = allgather_h(h2_loc, F2, 2)

            # ---- Layer 3 ----
            s3_tiles = build_support(h2_full, w3_sb, F2, D_OUT, "s3")
            psy3 = aggregate(s3_tiles, D_OUT)
            h3_sb = hpool.tile([D_OUT, ns], F32, tag="h3")
            relu_bias(psy3, b3_sb, D_OUT, h3_sb)

            # ---- out = relu(h3 + x) ----
            o_sb = hpool.tile([D_OUT, ns], F32, tag="osum")
            nc.vector.tensor_add(o_sb[:, :], h3_sb[:, :], xts_sb[:, :])
            r_sb = hpool.tile([D_OUT, ns], F32, tag="orelu")
            nc.vector.tensor_relu(r_sb[:, :], o_sb[:, :])
            nc.sync.dma_start(out=outT[:, :], in_=r_sb[:, :])

    nc.compile()
    return nc


def _ensure_ntff_hook():
    """Register the axon NTFF profile hook if the image's antenv lacks it.

    Mirrors trn_agent_boot.trn_boot._ntff_profile_via_ctypes — drives NRT
    profiling through libaxon_pjrt.so's C ABI so run_bass_kernel_spmd can
    capture exec_time_ns under axon. Only used when tracing is requested.
    """
    import contextlib
    import ctypes
    import sys
    import types

    try:
        from antenv.axon_hooks import get_axon_ntff_profile_hook  # noqa: F401

        return
    except ImportError:
        pass

    so_path = "/opt/axon/libaxon_pjrt.so"
    lib = ctypes.CDLL(so_path)
    if not hasattr(lib, "axon_start_nrt_profile"):
        return
    lib.axon_start_nrt_profile.argtypes = [
        ctypes.POINTER(ctypes.c_int64),
        ctypes.c_size_t,
    ]
    lib.axon_start_nrt_profile.restype = ctypes.c_int64
    lib.axon_stop_nrt_profile.argtypes = [ctypes.c_char_p]
    lib.axon_stop_nrt_profile.restype = ctypes.c_int64

    @contextlib.contextmanager
    def _hook(output_dir, device_ids):
        import jax

        jax.devices()
        if device_ids:
            ids = (ctypes.c_int64 * len(device_ids))(*device_ids)
            rc = lib.axon_start_nrt_profile(ids, len(device_ids))
        else:
            rc = lib.axon_start_nrt_profile(None, 0)
        if rc != 0:
            raise RuntimeError(f"axon_start_nrt_profile rc={rc}")
        try:
            yield
        finally:
            n = lib.axon_stop_nrt_profile(str(output_dir).encode())
            print(f"ntff profile: {n} file(s) written to {output_dir}")

    mod = types.ModuleType("antenv.axon_hooks")
    _state = {"hook": _hook}
    mod.get_axon_ntff_profile_hook = lambda: _state["hook"]
    mod.set_axon_ntff_profile_hook = lambda h: _state.update(hook=h)
    sys.modules["antenv.axon_hooks"] = mod
    import antenv

    antenv.axon_hooks = mod


_PROGRAM = None


def _get_program():
    global _PROGRAM
    if _PROGRAM is None:
        _PROGRAM = build_program()
    return _PROGRAM


def kernel(**inputs):
    global LAST_RESULTS
    x = np.asarray(inputs["x"], dtype=np.float32)
    adj = np.asarray(inputs["adj"], dtype=np.float32)
    np_adj_dt = mybir.dt.np(ADJ_DT)

    adjT = np.ascontiguousarray(adj.T).astype(np_adj_dt)
    xT = np.ascontiguousarray(x.T)
    base = {
        "xT": xT.astype(np_adj_dt),
        "W1": np.asarray(inputs["W1"], np.float32).astype(np_adj_dt),
        "W2": np.asarray(inputs["W2"], np.float32).astype(np_adj_dt),
        "W3": np.asarray(inputs["W3"], np.float32).astype(np_adj_dt),
        "b1": np.asarray(inputs["b1"], np.float32).reshape(F1, 1),
        "b2": np.asarray(inputs["b2"], np.float32).reshape(F2, 1),
        "b3": np.asarray(inputs["b3"], np.float32).reshape(D_OUT, 1),
    }
    in_maps = []
    for c in range(NCORES):
        sl = slice(c * NS, (c + 1) * NS)
        in_maps.append(
            dict(
                base,
                adjT=np.ascontiguousarray(adjT[:, sl]),
                xTs=np.ascontiguousarray(xT[:, sl]),
            )
        )

    nc = _get_program()
    trace = bool(int(os.environ.get("GCN_TRACE", "0")))
    extra = {}
    if trace:
        _ensure_ntff_hook()
        if os.environ.get("GCN_TRACE_DIR"):
            os.makedirs(os.environ["GCN_TRACE_DIR"], exist_ok=True)
            extra["tmpdir"] = os.environ["GCN_TRACE_DIR"]
    LAST_RESULTS = run_bass_kernel_spmd(
        nc, in_maps, list(range(NCORES)), trace=trace, **extra
    )
    out = np.concatenate(
        [np.asarray(LAST_RESULTS.results[c]["outT"]).T for c in range(NCORES)],
        axis=0,
    )
    return np.ascontiguousarray(out.astype(np.float32))


# revision 19
# speedup vs baseline: 1.4258x; 1.0080x over previous
"""3-layer GCN (Kipf GraphConvolution) on 8 Trainium2 NeuronCores.

Math per layer: h = relu(adj @ (h @ W) + b); final out = relu(h3 + x).

Strategy (row-shard / 1D node partition):
  - adj is pre-transposed on the host; core c gets adjT[:, c*NS:(c+1)*NS]
    (i.e. the rows of adj it owns, laid out contraction-major) in bf16.
  - On-chip, activations are kept TRANSPOSED: hT [F, nodes] with features on
    partitions. The aggregation y = adj @ s is computed as
    yT = sT_stationary.T-contracted with adjT_moving:
        yT[f, i] = sum_j s[j, f] * adjT[j, i]
    so adj streams through the PE as the moving operand (N cycles per
    128x512 tile) and is never transposed on chip.
  - s tiles (normal layout [j, f]) are built from hT via a second small
    matmul: s[j, g] = sum_f hT[f, j] * W[f, g]  (lhsT = hT slice).
  - Layer boundaries need full-graph support: AllGather of the local
    hT [F, NS] (100 KB f32) across the 8 cores, twice (after layers 1, 2).
  - Layer 1's support s1 = x @ W1 is computed redundantly on every core from
    the full xT (no AllGather needed).

Per-core HBM traffic is dominated by streaming the 10000x1250 adjT shard
three times (bf16: 3 x 25 MB).
"""

import math
import os

import numpy as np

import concourse.bacc as bacc
import concourse.mybir as mybir
import concourse.tile as tile
from concourse.bass_utils import run_bass_kernel_spmd

# Problem geometry (hardcoded per the harness contract).
N = 10000
D_IN = 128
F1 = 20
F2 = 20
D_OUT = 128
NCORES = 8
NS = N // NCORES  # 1250 nodes per core
NT = math.ceil(N / 128)  # 79 contraction tiles, last has 16 rows
ICHUNK = 512

F32 = mybir.dt.float32
ADJ_DT = mybir.dt.bfloat16  # dtype adj (and s tiles) are streamed/matmul'd in

# Filled by kernel() so a harness/test can inspect HW timing.
LAST_RESULTS = None


def _chunks(total, step):
    out = []
    i = 0
    while i < total:
        out.append((i, min(step, total - i)))
        i += step
    return out


def build_program(n=N, ncores=NCORES, adj_dt=ADJ_DT):
    ns = n // ncores
    nt = math.ceil(n / 128)
    chunks = _chunks(ns, ICHUNK)
    relu = mybir.ActivationFunctionType.Relu
    # adjT k-tiles are streamed in slabs of SLAB tiles per DMA (~640 KB each)
    SLAB = 2
    n_full_slabs = (n // 128) // SLAB  # full 4x128-row slabs
    nt_tail_start = n_full_slabs * SLAB  # remaining k-tiles loaded singly

    nc = bacc.Bacc("TRN2", target_bir_lowering=False, debug=False)

    adjT = nc.dram_tensor("adjT", [n, ns], adj_dt, kind="ExternalInput")
    xT = nc.dram_tensor("xT", [D_IN, n], adj_dt, kind="ExternalInput")
    xTs = nc.dram_tensor("xTs", [D_IN, ns], F32, kind="ExternalInput")
    W1 = nc.dram_tensor("W1", [D_IN, F1], adj_dt, kind="ExternalInput")
    W2 = nc.dram_tensor("W2", [F1, F2], adj_dt, kind="ExternalInput")
    W3 = nc.dram_tensor("W3", [F2, D_OUT], adj_dt, kind="ExternalInput")
    b1 = nc.dram_tensor("b1", [F1, 1], F32, kind="ExternalInput")
    b2 = nc.dram_tensor("b2", [F2, 1], F32, kind="ExternalInput")
    b3 = nc.dram_tensor("b3", [D_OUT, 1], F32, kind="ExternalInput")
    outT = nc.dram_tensor("outT", [D_OUT, ns], F32, kind="ExternalOutput")

    with tile.TileContext(nc, num_cores=ncores) as tc:
        with (
            tc.tile_pool(name="const", bufs=1) as const,
            tc.tile_pool(name="s", bufs=1) as spool,
            tc.tile_pool(name="h", bufs=1) as hpool,
            tc.tile_pool(name="adj", bufs=16) as adjpool,
            tc.tile_pool(name="psy", bufs=2, space="PSUM") as psy_pool,
            tc.tile_pool(name="pss", bufs=2, space="PSUM") as pss_pool,
            tc.tile_pool(name="dram", bufs=1, space="DRAM") as dpool,
        ):
            w1_sb = const.tile([D_IN, F1], adj_dt, tag="w1")
            w2_sb = const.tile([F1, F2], adj_dt, tag="w2")
            w3_sb = const.tile([F2, D_OUT], adj_dt, tag="w3")
            b1_sb = const.tile([F1, 1], F32, tag="b1")
            b2_sb = const.tile([F2, 1], F32, tag="b2")
            b3_sb = const.tile([D_OUT, 1], F32, tag="b3")
            xts_sb = const.tile([D_IN, ns], F32, tag="xts")
            for sb, dr in (
                (w1_sb, W1),
                (w2_sb, W2),
                (w3_sb, W3),
                (b1_sb, b1),
                (b2_sb, b2),
                (b3_sb, b3),
                (xts_sb, xTs),
            ):
                nc.gpsimd.dma_start(out=sb[:, :], in_=dr[:, :])

            def build_support(src_sb, w_sb, fin, fout, lname):
                """s[j, g] = sum_f src_T[f, j] * W[f, g], one tile per j-tile
                so the aggregation can consume tiles as they are built."""
                tiles = []
                for jt in range(nt):
                    m = min(128, n - jt * 128)
                    ps = pss_pool.tile([128, max(F1, D_OUT)], F32, tag="pss")
                    nc.tensor.matmul(
                        ps[:m, :fout],
                        lhsT=src_sb[:fin, jt * 128 : jt * 128 + m],
                        rhs=w_sb[:fin, :fout],
                        start=True,
                        stop=True,
                    )
                    st = spool.tile(
                        [128, fout], adj_dt, tag=f"{lname}_{jt}", name=f"{lname}_{jt}"
                    )
                    nc.vector.tensor_copy(st[:m, :], ps[:m, :fout])
                    tiles.append(st)
                return tiles

            def aggregate(s_tiles, fout):
                """yT[f, i] += s_tile.T @ adjT_tile over all contraction tiles.

                adjT is streamed in slabs of SLAB k-tiles per DMA (row-blocks
                [512, ns] loaded as [128, SLAB*ns] with the 128-row sub-blocks
                side by side in the free dim) so each DMA is ~1.25 MB.
                """

                def mm(kt, at_slice, k):
                    for ic, (i0, ilen) in enumerate(chunks):
                        nc.tensor.matmul(
                            psy[ic][:fout, :ilen],
                            lhsT=s_tiles[kt][:k, :fout],
                            rhs=at_slice[:k, i0 : i0 + ilen],
                            start=(kt == 0),
                            stop=(kt == nt - 1),
                        )

                psy = [
                    psy_pool.tile(
                        [128, ICHUNK], F32, tag=f"psy{ic}", name=f"psy{ic}"
                    )
                    for ic in range(len(chunks))
                ]
                for sl in range(n_full_slabs):
                    at = adjpool.tile([128, SLAB * ns], adj_dt, tag="adjstream")
                    r0 = sl * SLAB * 128
                    nc.sync.dma_start(
                        out=at[:, :].rearrange("p (a i) -> p a i", a=SLAB),
                        in_=adjT[r0 : r0 + SLAB * 128, :].rearrange(
                            "(a p) i -> p a i", p=128
                        ),
                    )
                    for a in range(SLAB):
                        mm(sl * SLAB + a, at[:, a * ns : (a + 1) * ns], 128)
                for kt in range(nt_tail_start, nt):
                    k = min(128, n - kt * 128)
                    at = adjpool.tile(
                        [128, SLAB * ns], adj_dt, tag="adjstream", name="at_tail"
                    )
                    nc.sync.dma_start(
                        out=at[:k, :ns], in_=adjT[kt * 128 : kt * 128 + k, :]
                    )
                    mm(kt, at[:, :ns], k)
                return psy

            def relu_bias(psy, b_sb, fout, dst_sb):
                for ic, (i0, ilen) in enumerate(chunks):
                    nc.scalar.activation(
                        dst_sb[:fout, i0 : i0 + ilen],
                        psy[ic][:fout, :ilen],
                        relu,
                        bias=b_sb[:fout, :],
                    )

            def allgather_h(h_loc, f, layer):
                cc_in = dpool.tile([f, ns], adj_dt, tag=f"ccin{layer}")
                cc_out = dpool.tile([ncores * f, ns], adj_dt, tag=f"ccout{layer}")
                nc.gpsimd.dma_start(out=cc_in[:, :], in_=h_loc[:, :])
                nc.gpsimd.collective_compute(
                    "AllGather",
                    mybir.AluOpType.bypass,
                    replica_groups=[list(range(ncores))],
                    ins=[cc_in.opt()],
                    outs=[cc_out.opt()],
                )
                h_full = hpool.tile([f, n], adj_dt, tag="hfull", name=f"hfull{layer}")
                nc.gpsimd.dma_start(
                    out=h_full[:, :].rearrange("p (r i) -> p r i", r=ncores),
                    in_=cc_out[:, :].rearrange("(r p) i -> p r i", p=f),
                )
                return h_full

            # ---- Layer 1: s1 = x @ W1 built redundantly from full xT ----
            xt_sb = const.tile([D_IN, n], adj_dt, tag="xt")
            nc.gpsimd.dma_start(out=xt_sb[:, :], in_=xT[:, :])
            s1_tiles = build_support(xt_sb, w1_sb, D_IN, F1, "s1")
            psy1 = aggregate(s1_tiles, F1)
            h1_loc = hpool.tile([F1, ns], adj_dt, tag="hloc1")
            relu_bias(psy1, b1_sb, F1, h1_loc)
            h1_full = allgather_h(h1_loc, F1, 1)

            # ---- Layer 2 ----
            s2_tiles = build_support(h1_full, w2_sb, F1, F2, "s2")
            psy2 = aggregate(s2_tiles, F2)
            h2_loc = hpool.tile([F2, ns], adj_dt, tag="hloc2")
            relu_bias(psy2, b2_sb, F2, h2_loc)
            h2_full = allgather_h(h2_loc, F2, 2)

            # ---- Layer 3 ----
            s3_tiles = build_support(h2_full, w3_sb, F2, D_OUT, "s3")
            psy3 = aggregate(s3_tiles, D_OUT)
            h3_sb = hpool.tile([D_OUT, ns], F32, tag="h3")
            relu_bias(psy3, b3_sb, D_OUT, h3_sb)

            # ---- out = relu(h3 + x) ----
            o_sb = hpool.tile([D_OUT, ns], F32, tag="osum")
            nc.vector.tensor_add(o_sb[:, :], h3_sb[:, :], xts_sb[:, :])
            r_sb = hpool.tile([D_OUT, ns], F32, tag="orelu")
            nc.vector.tensor_relu(r_sb[:, :], o_sb[:, :])
            nc.sync.dma_start(out=outT[:, :], in_=r_sb[:, :])

    nc.compile()
    return nc


def _ensure_ntff_hook():
    """Register the axon NTFF profile hook if the image's antenv lacks it.

    Mirrors trn_agent_boot.trn_boot._ntff_profile_via_ctypes — drives NRT
    profiling through libaxon_pjrt.so's C ABI so run_bass_kernel_spmd can
    capture exec_time_ns under axon. Only used when tracing is requested.
    """
    import contextlib
    import ctypes
    import sys
    import types

    try:
        from antenv.axon_hooks import get_axon_ntff_profile_hook  # noqa: F401

        return
    except ImportError:
        pass

    so_path = "/opt/axon/libaxon_pjrt.so"
    lib = ctypes.CDLL(so_path)
    if not hasattr(lib, "axon_start_nrt_profile"):
        return
    lib.axon_start_nrt_profile.argtypes = [
        ctypes.POINTER(ctypes.c_int64),
        ctypes.c_size_t,
    ]
    lib.axon_start_nrt_profile.restype = ctypes.c_int64
    lib.axon_stop_nrt_profile.argtypes = [ctypes.c_char_p]
    lib.axon_stop_nrt_profile.restype = ctypes.c_int64

    @contextlib.contextmanager
    def _hook(output_dir, device_ids):
        import jax

        jax.devices()
        if device_ids:
            ids = (ctypes.c_int64 * len(device_ids))(*device_ids)
            rc = lib.axon_start_nrt_profile(ids, len(device_ids))
        else:
            rc = lib.axon_start_nrt_profile(None, 0)
        if rc != 0:
            raise RuntimeError(f"axon_start_nrt_profile rc={rc}")
        try:
            yield
        finally:
            n = lib.axon_stop_nrt_profile(str(output_dir).encode())
            print(f"ntff profile: {n} file(s) written to {output_dir}")

    mod = types.ModuleType("antenv.axon_hooks")
    _state = {"hook": _hook}
    mod.get_axon_ntff_profile_hook = lambda: _state["hook"]
    mod.set_axon_ntff_profile_hook = lambda h: _state.update(hook=h)
    sys.modules["antenv.axon_hooks"] = mod
    import antenv

    antenv.axon_hooks = mod


_PROGRAM = None


def _get_program():
    global _PROGRAM
    if _PROGRAM is None:
        _PROGRAM = build_program()
    return _PROGRAM


def kernel(**inputs):
    global LAST_RESULTS
    x = np.asarray(inputs["x"], dtype=np.float32)
    adj = np.asarray(inputs["adj"], dtype=np.float32)
    np_adj_dt = mybir.dt.np(ADJ_DT)

    adjT = np.ascontiguousarray(adj.T).astype(np_adj_dt)
    xT = np.ascontiguousarray(x.T)
    base = {
        "xT": xT.astype(np_adj_dt),
        "W1": np.asarray(inputs["W1"], np.float32).astype(np_adj_dt),
        "W2": np.asarray(inputs["W2"], np.float32).astype(np_adj_dt),
        "W3": np.asarray(inputs["W3"], np.float32).astype(np_adj_dt),
        "b1": np.asarray(inputs["b1"], np.float32).reshape(F1, 1),
        "b2": np.asarray(inputs["b2"], np.float32).reshape(F2, 1),
        "b3": np.asarray(inputs["b3"], np.float32).reshape(D_OUT, 1),
    }
    in_maps = []
    for c in range(NCORES):
        sl = slice(c * NS, (c + 1) * NS)
        in_maps.append(
            dict(
                base,
                adjT=np.ascontiguousarray(adjT[:, sl]),
                xTs=np.ascontiguousarray(xT[:, sl]),
            )
        )

    nc = _get_program()
    trace = bool(int(os.environ.get("GCN_TRACE", "0")))
    extra = {}
    if trace:
        _ensure_ntff_hook()
        if os.environ.get("GCN_TRACE_DIR"):
            os.makedirs(os.environ["GCN_TRACE_DIR"], exist_ok=True)
            extra["tmpdir"] = os.environ["GCN_TRACE_DIR"]
    LAST_RESULTS = run_bass_kernel_spmd(
        nc, in_maps, list(range(NCORES)), trace=trace, **extra
    )
    out = np.concatenate(
        [np.asarray(LAST_RESULTS.results[c]["outT"]).T for c in range(NCORES)],
        axis=0,
    )
    return np.ascontiguousarray(out.astype(np.float32))


# revision 20
# speedup vs baseline: 1.4363x; 1.0074x over previous
"""3-layer GCN (Kipf GraphConvolution) on 8 Trainium2 NeuronCores.

Math per layer: h = relu(adj @ (h @ W) + b); final out = relu(h3 + x).

Strategy (row-shard / 1D node partition):
  - adj is pre-transposed on the host; core c gets adjT[:, c*NS:(c+1)*NS]
    (i.e. the rows of adj it owns, laid out contraction-major) in bf16.
  - On-chip, activations are kept TRANSPOSED: hT [F, nodes] with features on
    partitions. The aggregation y = adj @ s is computed as
    yT = sT_stationary.T-contracted with adjT_moving:
        yT[f, i] = sum_j s[j, f] * adjT[j, i]
    so adj streams through the PE as the moving operand (N cycles per
    128x512 tile) and is never transposed on chip.
  - s tiles (normal layout [j, f]) are built from hT via a second small
    matmul: s[j, g] = sum_f hT[f, j] * W[f, g]  (lhsT = hT slice).
  - Layer boundaries need full-graph support: AllGather of the local
    hT [F, NS] (100 KB f32) across the 8 cores, twice (after layers 1, 2).
  - Layer 1's support s1 = x @ W1 is computed redundantly on every core from
    the full xT (no AllGather needed).

Per-core HBM traffic is dominated by streaming the 10000x1250 adjT shard
three times (bf16: 3 x 25 MB).
"""

import math
import os

import numpy as np

import concourse.bacc as bacc
import concourse.mybir as mybir
import concourse.tile as tile
from concourse.bass_utils import run_bass_kernel_spmd

# Problem geometry (hardcoded per the harness contract).
N = 10000
D_IN = 128
F1 = 20
F2 = 20
D_OUT = 128
NCORES = 8
NS = N // NCORES  # 1250 nodes per core
NT = math.ceil(N / 128)  # 79 contraction tiles, last has 16 rows
ICHUNK = 512

F32 = mybir.dt.float32
ADJ_DT = mybir.dt.bfloat16  # dtype adj (and s tiles) are streamed/matmul'd in

# Filled by kernel() so a harness/test can inspect HW timing.
LAST_RESULTS = None


def _chunks(total, step):
    out = []
    i = 0
    while i < total:
        out.append((i, min(step, total - i)))
        i += step
    return out


def build_program(n=N, ncores=NCORES, adj_dt=ADJ_DT):
    ns = n // ncores
    nt = math.ceil(n / 128)
    chunks = _chunks(ns, ICHUNK)
    relu = mybir.ActivationFunctionType.Relu
    # adjT k-tiles are streamed in slabs of SLAB tiles per DMA (~640 KB each)
    SLAB = 2
    n_full_slabs = (n // 128) // SLAB  # full 4x128-row slabs
    nt_tail_start = n_full_slabs * SLAB  # remaining k-tiles loaded singly

    nc = bacc.Bacc("TRN2", target_bir_lowering=False, debug=False)

    adjT = nc.dram_tensor("adjT", [n, ns], adj_dt, kind="ExternalInput")
    xT = nc.dram_tensor("xT", [D_IN, n], adj_dt, kind="ExternalInput")
    xTs = nc.dram_tensor("xTs", [D_IN, ns], F32, kind="ExternalInput")
    W1 = nc.dram_tensor("W1", [D_IN, F1], adj_dt, kind="ExternalInput")
    W2 = nc.dram_tensor("W2", [F1, F2], adj_dt, kind="ExternalInput")
    W3 = nc.dram_tensor("W3", [F2, D_OUT], adj_dt, kind="ExternalInput")
    b1 = nc.dram_tensor("b1", [F1, 1], F32, kind="ExternalInput")
    b2 = nc.dram_tensor("b2", [F2, 1], F32, kind="ExternalInput")
    b3 = nc.dram_tensor("b3", [D_OUT, 1], F32, kind="ExternalInput")
    outT = nc.dram_tensor("outT", [D_OUT, ns], F32, kind="ExternalOutput")

    with tile.TileContext(nc, num_cores=ncores) as tc:
        with (
            tc.tile_pool(name="const", bufs=1) as const,
            tc.tile_pool(name="s", bufs=1) as spool,
            tc.tile_pool(name="h", bufs=1) as hpool,
            tc.tile_pool(name="adj", bufs=16) as adjpool,
            tc.tile_pool(name="psy", bufs=2, space="PSUM") as psy_pool,
            tc.tile_pool(name="pss", bufs=2, space="PSUM") as pss_pool,
            tc.tile_pool(name="dram", bufs=1, space="DRAM") as dpool,
        ):
            w1_sb = const.tile([D_IN, F1], adj_dt, tag="w1")
            w2_sb = const.tile([F1, F2], adj_dt, tag="w2")
            w3_sb = const.tile([F2, D_OUT], adj_dt, tag="w3")
            b1_sb = const.tile([F1, 1], F32, tag="b1")
            b2_sb = const.tile([F2, 1], F32, tag="b2")
            b3_sb = const.tile([D_OUT, 1], F32, tag="b3")
            xts_sb = const.tile([D_IN, ns], F32, tag="xts")
            for sb, dr in (
                (w1_sb, W1),
                (w2_sb, W2),
                (w3_sb, W3),
                (b1_sb, b1),
                (b2_sb, b2),
                (b3_sb, b3),
                (xts_sb, xTs),
            ):
                nc.gpsimd.dma_start(out=sb[:, :], in_=dr[:, :])

            # Tiny warm-up AllGather: pays the first-collective ncfw wake-up
            # cost (~10 us) concurrently with layer-1 streaming so the real
            # AllGathers trigger with ~1 us delay.
            warm_in = dpool.tile([1, 4], F32, tag="warmin")
            warm_out = dpool.tile([ncores, 4], F32, tag="warmout")
            nc.gpsimd.collective_compute(
                "AllGather",
                mybir.AluOpType.bypass,
                replica_groups=[list(range(ncores))],
                ins=[warm_in.opt()],
                outs=[warm_out.opt()],
            )

            def build_support(src_sb, w_sb, fin, fout, lname):
                """s[j, g] = sum_f src_T[f, j] * W[f, g], one tile per j-tile
                so the aggregation can consume tiles as they are built."""
                tiles = []
                for jt in range(nt):
                    m = min(128, n - jt * 128)
                    ps = pss_pool.tile([128, max(F1, D_OUT)], F32, tag="pss")
                    nc.tensor.matmul(
                        ps[:m, :fout],
                        lhsT=src_sb[:fin, jt * 128 : jt * 128 + m],
                        rhs=w_sb[:fin, :fout],
                        start=True,
                        stop=True,
                    )
                    st = spool.tile(
                        [128, fout], adj_dt, tag=f"{lname}_{jt}", name=f"{lname}_{jt}"
                    )
                    nc.vector.tensor_copy(st[:m, :], ps[:m, :fout])
                    tiles.append(st)
                return tiles

            def aggregate(s_tiles, fout):
                """yT[f, i] += s_tile.T @ adjT_tile over all contraction tiles.

                adjT is streamed in slabs of SLAB k-tiles per DMA (row-blocks
                [512, ns] loaded as [128, SLAB*ns] with the 128-row sub-blocks
                side by side in the free dim) so each DMA is ~1.25 MB.
                """

                def mm(kt, at_slice, k):
                    for ic, (i0, ilen) in enumerate(chunks):
                        nc.tensor.matmul(
                            psy[ic][:fout, :ilen],
                            lhsT=s_tiles[kt][:k, :fout],
                            rhs=at_slice[:k, i0 : i0 + ilen],
                            start=(kt == 0),
                            stop=(kt == nt - 1),
                        )

                psy = [
                    psy_pool.tile(
                        [128, ICHUNK], F32, tag=f"psy{ic}", name=f"psy{ic}"
                    )
                    for ic in range(len(chunks))
                ]
                for sl in range(n_full_slabs):
                    at = adjpool.tile([128, SLAB * ns], adj_dt, tag="adjstream")
                    r0 = sl * SLAB * 128
                    nc.sync.dma_start(
                        out=at[:, :].rearrange("p (a i) -> p a i", a=SLAB),
                        in_=adjT[r0 : r0 + SLAB * 128, :].rearrange(
                            "(a p) i -> p a i", p=128
                        ),
                    )
                    for a in range(SLAB):
                        mm(sl * SLAB + a, at[:, a * ns : (a + 1) * ns], 128)
                for kt in range(nt_tail_start, nt):
                    k = min(128, n - kt * 128)
                    at = adjpool.tile(
                        [128, SLAB * ns], adj_dt, tag="adjstream", name="at_tail"
                    )
                    nc.sync.dma_start(
                        out=at[:k, :ns], in_=adjT[kt * 128 : kt * 128 + k, :]
                    )
                    mm(kt, at[:, :ns], k)
                return psy

            def relu_bias(psy, b_sb, fout, dst_sb):
                for ic, (i0, ilen) in enumerate(chunks):
                    nc.scalar.activation(
                        dst_sb[:fout, i0 : i0 + ilen],
                        psy[ic][:fout, :ilen],
                        relu,
                        bias=b_sb[:fout, :],
                    )

            def allgather_h(h_loc, f, layer):
                cc_in = dpool.tile([f, ns], adj_dt, tag=f"ccin{layer}")
                cc_out = dpool.tile([ncores * f, ns], adj_dt, tag=f"ccout{layer}")
                nc.gpsimd.dma_start(out=cc_in[:, :], in_=h_loc[:, :])
                nc.gpsimd.collective_compute(
                    "AllGather",
                    mybir.AluOpType.bypass,
                    replica_groups=[list(range(ncores))],
                    ins=[cc_in.opt()],
                    outs=[cc_out.opt()],
                )
                h_full = hpool.tile([f, n], adj_dt, tag="hfull", name=f"hfull{layer}")
                nc.gpsimd.dma_start(
                    out=h_full[:, :].rearrange("p (r i) -> p r i", r=ncores),
                    in_=cc_out[:, :].rearrange("(r p) i -> p r i", p=f),
                )
                return h_full

            # ---- Layer 1: s1 = x @ W1 built redundantly from full xT ----
            xt_sb = const.tile([D_IN, n], adj_dt, tag="xt")
            nc.gpsimd.dma_start(out=xt_sb[:, :], in_=xT[:, :])
            s1_tiles = build_support(xt_sb, w1_sb, D_IN, F1, "s1")
            psy1 = aggregate(s1_tiles, F1)
            h1_loc = hpool.tile([F1, ns], adj_dt, tag="hloc1")
            relu_bias(psy1, b1_sb, F1, h1_loc)
            h1_full = allgather_h(h1_loc, F1, 1)

            # ---- Layer 2 ----
            s2_tiles = build_support(h1_full, w2_sb, F1, F2, "s2")
            psy2 = aggregate(s2_tiles, F2)
            h2_loc = hpool.tile([F2, ns], adj_dt, tag="hloc2")
            relu_bias(psy2, b2_sb, F2, h2_loc)
            h2_full = allgather_h(h2_loc, F2, 2)

            # ---- Layer 3 ----
            s3_tiles = build_support(h2_full, w3_sb, F2, D_OUT, "s3")
            psy3 = aggregate(s3_tiles, D_OUT)
            h3_sb = hpool.tile([D_OUT, ns], F32, tag="h3")
            relu_bias(psy3, b3_sb, D_OUT, h3_sb)

            # ---- out = relu(h3 + x) ----
            o_sb = hpool.tile([D_OUT, ns], F32, tag="osum")
            nc.vector.tensor_add(o_sb[:, :], h3_sb[:, :], xts_sb[:, :])
            r_sb = hpool.tile([D_OUT, ns], F32, tag="orelu")
            nc.vector.tensor_relu(r_sb[:, :], o_sb[:, :])
            nc.sync.dma_start(out=outT[:, :], in_=r_sb[:, :])

    nc.compile()
    return nc


def _ensure_ntff_hook():
    """Register the axon NTFF profile hook if the image's antenv lacks it.

    Mirrors trn_agent_boot.trn_boot._ntff_profile_via_ctypes — drives NRT
    profiling through libaxon_pjrt.so's C ABI so run_bass_kernel_spmd can
    capture exec_time_ns under axon. Only used when tracing is requested.
    """
    import contextlib
    import ctypes
    import sys
    import types

    try:
        from antenv.axon_hooks import get_axon_ntff_profile_hook  # noqa: F401

        return
    except ImportError:
        pass

    so_path = "/opt/axon/libaxon_pjrt.so"
    lib = ctypes.CDLL(so_path)
    if not hasattr(lib, "axon_start_nrt_profile"):
        return
    lib.axon_start_nrt_profile.argtypes = [
        ctypes.POINTER(ctypes.c_int64),
        ctypes.c_size_t,
    ]
    lib.axon_start_nrt_profile.restype = ctypes.c_int64
    lib.axon_stop_nrt_profile.argtypes = [ctypes.c_char_p]
    lib.axon_stop_nrt_profile.restype = ctypes.c_int64

    @contextlib.contextmanager
    def _hook(output_dir, device_ids):
        import jax

        jax.devices()
        if device_ids:
            ids = (ctypes.c_int64 * len(device_ids))(*device_ids)
            rc = lib.axon_start_nrt_profile(ids, len(device_ids))
        else:
            rc = lib.axon_start_nrt_profile(None, 0)
        if rc != 0:
            raise RuntimeError(f"axon_start_nrt_profile rc={rc}")
        try:
            yield
        finally:
            n = lib.axon_stop_nrt_profile(str(output_dir).encode())
            print(f"ntff profile: {n} file(s) written to {output_dir}")

    mod = types.ModuleType("antenv.axon_hooks")
    _state = {"hook": _hook}
    mod.get_axon_ntff_profile_hook = lambda: _state["hook"]
    mod.set_axon_ntff_profile_hook = lambda h: _state.update(hook=h)
    sys.modules["antenv.axon_hooks"] = mod
    import antenv

    antenv.axon_hooks = mod


_PROGRAM = None


def _get_program():
    global _PROGRAM
    if _PROGRAM is None:
        _PROGRAM = build_program()
    return _PROGRAM


def kernel(**inputs):
    global LAST_RESULTS
    x = np.asarray(inputs["x"], dtype=np.float32)
    adj = np.asarray(inputs["adj"], dtype=np.float32)
    np_adj_dt = mybir.dt.np(ADJ_DT)

    adjT = np.ascontiguousarray(adj.T).astype(np_adj_dt)
    xT = np.ascontiguousarray(x.T)
    base = {
        "xT": xT.astype(np_adj_dt),
        "W1": np.asarray(inputs["W1"], np.float32).astype(np_adj_dt),
        "W2": np.asarray(inputs["W2"], np.float32).astype(np_adj_dt),
        "W3": np.asarray(inputs["W3"], np.float32).astype(np_adj_dt),
        "b1": np.asarray(inputs["b1"], np.float32).reshape(F1, 1),
        "b2": np.asarray(inputs["b2"], np.float32).reshape(F2, 1),
        "b3": np.asarray(inputs["b3"], np.float32).reshape(D_OUT, 1),
    }
    in_maps = []
    for c in range(NCORES):
        sl = slice(c * NS, (c + 1) * NS)
        in_maps.append(
            dict(
                base,
                adjT=np.ascontiguousarray(adjT[:, sl]),
                xTs=np.ascontiguousarray(xT[:, sl]),
            )
        )

    nc = _get_program()
    trace = bool(int(os.environ.get("GCN_TRACE", "0")))
    extra = {}
    if trace:
        _ensure_ntff_hook()
        if os.environ.get("GCN_TRACE_DIR"):
            os.makedirs(os.environ["GCN_TRACE_DIR"], exist_ok=True)
            extra["tmpdir"] = os.environ["GCN_TRACE_DIR"]
    LAST_RESULTS = run_bass_kernel_spmd(
        nc, in_maps, list(range(NCORES)), trace=trace, **extra
    )
    out = np.concatenate(
        [np.asarray(LAST_RESULTS.results[c]["outT"]).T for c in range(NCORES)],
        axis=0,
    )
    return np.ascontiguousarray(out.astype(np.float32))
